# revision 11
# baseline (speedup 1.0000x reference)
"""Multi-head attention (B=4, S=2048, E=768, H=12) on 8 NeuronCores.

Sharding: core c handles batch c//2 and head-group c%2 (6 heads = 3 pairs).
Each core computes its heads' attention plus a partial output projection
(E-dim split); partials are summed on the host and bias added there.

Device-side layout (per core):
  - x^T [768, 2048] streamed in; QKV projection done with W as the
    stationary operand so Q^T/K^T come out in [d, s] layout with a head
    PAIR stacked on partition halves (head A -> partitions 0-63,
    head B -> 64-127).
  - Scores computed transposed (ST[k, q] = K Q^T) with row-tiled K=64
    matmuls; the two heads of a pair occupy PE row-halves 0-63/64-127
    and execute CONCURRENTLY (measured 185ns/mm vs 499ns serial).
  - exp on the scalar engine (PSUM -> SBUF fp16, fused 1/sqrt(64)
    scale), no max subtraction (scores ~N(0,1); exp cannot overflow).
  - P^T stays in [k, q] layout so A@V runs as V^T P^T with full K=128
    contraction; V carries an extra ones column so the softmax
    denominator accumulates in PSUM row 64 for free.
  - Normalize via DVE reciprocal_approx_fast (the plain DVE reciprocal
    is ~6 cyc/elem on ONE partition = 6.5us per chunk), gpsimd
    partition_broadcast + gpsimd mul (keeps PE/DVE free).
  - Output projection accumulated over the 3 pairs.

All matmul operands are fp16 (1 cycle/row on the PE like bf16, vs
fp32r which needs ap>=256 and runs hotter; 10 mantissa bits keep the
whole-pipeline rel err at ~7e-4). PSUM accumulation is fp32. fp16 also
halves the input DMA (compute starts earlier) and SBUF footprint.

The scalar engine (softmax exp, ~214us/core) and the PE (~250us/core)
are nearly balanced, so QKV-projection and output-projection matmuls
are emitted as fine-grained filler units inside the attention kt-loop
to keep the PE queue fed while the exp stream runs.
"""

import numpy as np

EMBED = 768
HEADS = 12
HD = 64
B = 4
S = 2048
N_CORES = 8
HPC = 6      # heads per core
PAIRS = 3    # head pairs per core
EKT = EMBED // 128   # 6 contraction tiles over E
SKT = S // 128       # 16 key tiles
QC = 512             # q-chunk (matmul free dim)
NQC = S // QC        # 4 q-chunks

_CACHE = {}
LAST_RESULTS = None  # stashed BassKernelResults for test harnesses


def _build(repeat=1):
    # repeat>1 wraps the body in a hardware loop -- used only by timing
    # harnesses to amplify exec time above host-side dispatch noise.
    import contextlib
    import concourse.bacc as bacc
    import concourse.tile as tile
    from concourse import mybir

    F16 = mybir.dt.float16
    F32 = mybir.dt.float32
    Exp = mybir.ActivationFunctionType.Exp

    nc = bacc.Bacc(None, target_bir_lowering=False)

    xt_d = nc.dram_tensor("xt", [EMBED, S], F16, kind="ExternalInput")
    wq_d = nc.dram_tensor("wq", [EMBED, HPC * HD], F16, kind="ExternalInput")
    wk_d = nc.dram_tensor("wk", [EMBED, HPC * HD], F16, kind="ExternalInput")
    wv_d = nc.dram_tensor("wv", [EMBED, HPC * HD], F16, kind="ExternalInput")
    wp_d = nc.dram_tensor("wp", [HPC * HD, EMBED], F16, kind="ExternalInput")
    o_d = nc.dram_tensor("o", [EMBED, S], F16, kind="ExternalOutput")

    with tile.TileContext(nc) as tc:
        with tc.tile_pool(name="w", bufs=1) as wpool, \
             tc.tile_pool(name="big", bufs=7) as big, \
             tc.tile_pool(name="v", bufs=1) as vpool, \
             tc.tile_pool(name="pt", bufs=6) as ptp, \
             tc.tile_pool(name="nrm", bufs=2) as nrm, \
             tc.tile_pool(name="mm_ps", bufs=2, space="PSUM") as mm_ps, \
             tc.tile_pool(name="st_ps", bufs=2, space="PSUM") as st_ps, \
             tc.tile_pool(name="av_ps", bufs=1, space="PSUM") as av_ps:

            # ---- resident inputs. DMA order = need order: pair-0 Q/K
            #      weights and the first x^T chunk come first so the
            #      first QKV matmuls start ~3us in; the rest streams
            #      behind them. ----
            wq = wpool.tile([128, EKT, HPC * HD], F16)
            wk = wpool.tile([128, EKT, HPC * HD], F16)
            wv = wpool.tile([128, EKT, HPC * HD], F16)
            xts = [wpool.tile([128, S], F16, name=f"xt{e}", tag=f"xt{e}")
                   for e in range(EKT)]

            def dma_w(dst, src_d, p):
                blk = slice(p * 128, (p + 1) * 128)
                nc.sync.dma_start(
                    dst[:, :, blk],
                    src_d.rearrange("(t p) m -> p t m", p=128)[:, :, blk])

            def dma_xt(c):
                for e in range(EKT):
                    nc.sync.dma_start(
                        xts[e][:, c * QC:(c + 1) * QC],
                        xt_d[e * 128:(e + 1) * 128, c * QC:(c + 1) * QC])

            dma_w(wq, wq_d, 0)
            dma_w(wk, wk_d, 0)
            dma_xt(0)
            nc.sync.dma_start(wv[:], wv_d.rearrange("(t p) m -> p t m", p=128))
            dma_xt(1)
            dma_xt(2)
            dma_xt(3)
            for p in range(1, PAIRS):
                dma_w(wq, wq_d, p)
                dma_w(wk, wk_d, p)
            wp = wpool.tile([128, PAIRS, EMBED], F16)
            nc.sync.dma_start(wp[:], wp_d.rearrange("(t p) e -> p t e", p=128))

            # V in [s, d] layout padded to a [head, 128] stationary:
            # col 0 = ones (AV row 0 = softmax denominator -- it must
            # land on PSUM partition 0 because reciprocal_approx_fast's
            # custom DVE ucode ignores the input AP's partition offset),
            # cols 1-63 = zeros (partition bases must be 0/32/64/96, so
            # the values go at cols 64-127 -> AV rows 64-127), cols
            # 64-127 = V. Matmul cost only depends on the moving free
            # size, so the padding is free on the PE.
            v_sb = [vpool.tile([128, HPC, 128], F16, name=f"v{st}",
                               tag=f"v{st}") for st in range(SKT)]
            ones = vpool.tile([128, 1], F16)
            nc.vector.memset(ones[:], 1.0)
            for st in range(SKT):
                nc.vector.memset(v_sb[st][:, :, 1:HD], 0.0)
                nc.vector.tensor_copy(
                    v_sb[st][:, :, 0:1],
                    ones[:, None, :].broadcast_to([128, HPC, 1]))

            rep_ctx = (tc.For_i(0, repeat, 1) if repeat > 1
                       else contextlib.nullcontext())
            rep_ctx.__enter__()

            qts = [None] * PAIRS
            kts = [None] * PAIRS
            ons = [None] * PAIRS

            def alloc_qk(p):
                if qts[p] is None:
                    qts[p] = big.tile([128, S], F16, tag="big", name=f"qt{p}")
                    kts[p] = big.tile([128, S], F16, tag="big", name=f"kt{p}")

            def emit_qk_unit(p, c, which):
                """One projection unit: 6 matmuls + copyback for Q or K,
                pair p, sequence chunk c."""
                alloc_qk(p)
                w_sb, dst = (wq, qts[p]) if which == "q" else (wk, kts[p])
                ps = mm_ps.tile([128, QC], F32, tag="mm")
                for e in range(EKT):
                    nc.tensor.matmul(
                        ps[:], w_sb[:, e, p * 128:(p + 1) * 128],
                        xts[e][:, c * QC:(c + 1) * QC],
                        start=(e == 0), stop=(e == EKT - 1))
                nc.vector.tensor_copy(dst[:, c * QC:(c + 1) * QC], ps[:])

            def emit_v_unit(st):
                """V for s-tile st, all 6 heads."""
                ps = mm_ps.tile([128, HPC * HD], F32, tag="mm")
                for e in range(EKT):
                    nc.tensor.matmul(
                        ps[:], xts[e][:, st * 128:(st + 1) * 128], wv[:, e, :],
                        start=(e == 0), stop=(e == EKT - 1))
                nc.vector.tensor_copy(
                    v_sb[st][:, :, HD:2 * HD],
                    ps[:].rearrange("p (h d) -> p h d", h=HPC))

            def emit_proj_unit(qc, et):
                """Output projection for q-chunk qc, one e-tile."""
                ps = mm_ps.tile([128, QC], F32, tag="mm")
                for p in range(PAIRS):
                    nc.tensor.matmul(
                        ps[:], wp[:, p, et * 128:(et + 1) * 128],
                        ons[p][:, qc * QC:(qc + 1) * QC],
                        start=(p == 0), stop=(p == PAIRS - 1))
                o_sb = nrm.tile([128, QC], F16, tag="o_sb", bufs=2)
                nc.vector.tensor_copy(o_sb[:], ps[:])
                nc.sync.dma_start(
                    o_d[et * 128:(et + 1) * 128, qc * QC:(qc + 1) * QC], o_sb[:])

            def emit_attn_qc(p, qc, fillers):
                """One q-chunk (512) of attention for head pair p.

                `fillers` is a list of zero-arg emission callbacks (QKV or
                proj units for other pairs) spread across the kt loop so
                the PE/DVE queues never hold a long serial run while the
                scalar engine streams exps.
                """
                av = av_ps.tile([128, 2, QC], F32, tag="av")
                AV_LAG = 4  # emit AV(kt-LAG) after ST(kt) so a blocked AV
                #             (av WAR on the previous chunk's normalize)
                #             never starves the exp stream of fresh STs
                pts = {}

                def emit_av(kt):
                    pt = pts.pop(kt)
                    for h in range(2):
                        nc.tensor.matmul(
                            av[:, h, :], v_sb[kt][:, 2 * p + h, :],
                            pt[:, h * QC:(h + 1) * QC],
                            start=(kt == 0), stop=(kt == SKT - 1))

                for kt in range(SKT):
                    st = st_ps.tile([128, 2 * QC], F32, tag="st")
                    for h in range(2):
                        nc.tensor.matmul(
                            st[:, h * QC:(h + 1) * QC],
                            kts[p][h * 64:(h + 1) * 64, kt * 128:(kt + 1) * 128],
                            qts[p][h * 64:(h + 1) * 64, qc * QC:(qc + 1) * QC],
                            start=True, stop=True)
                    pt = ptp.tile([128, 2 * QC], F16, tag="pt")
                    nc.scalar.activation(pt[:], st[:], Exp, scale=float(HD) ** -0.5)
                    pts[kt] = pt
                    if kt >= AV_LAG:
                        emit_av(kt - AV_LAG)
                    if kt < len(fillers):
                        fillers[kt]()
                for kt in range(SKT - AV_LAG, SKT):
                    emit_av(kt)
                if ons[p] is None:
                    ons[p] = big.tile([128, S], F16, tag="big", name=f"on{p}")
                # single copy frees the av accumulator; the rest of the
                # normalize chain runs off-PSUM without blocking the next
                # chunk's AV matmuls
                av_sb = nrm.tile([128, 2, QC], F32, tag="av_sb", bufs=1)
                nc.vector.tensor_copy(av_sb[:], av[:])
                recip = nrm.tile([1, 2, QC], F32, tag="recip")
                nc.vector.reciprocal_approx_fast(
                    recip[0:1, :, :], av_sb[0:1, :, :])
                # broadcast to all 128 partitions so the mul's two SBUF
                # inputs share base partition 64 (TensorTensor requires
                # equal input bases)
                bc = nrm.tile([128, 2, QC], F32, tag="bc")
                nc.gpsimd.partition_broadcast(bc[:], recip[0:1, :, :],
                                              channels=128)
                with nc.allow_low_precision(reason="fp16 normalized attn out"):
                    for h in range(2):
                        nc.gpsimd.tensor_mul(
                            ons[p][h * 64:(h + 1) * 64, qc * QC:(qc + 1) * QC],
                            av_sb[64:128, h, :], bc[64:128, h, :])

            # ---- emission schedule ----
            # Correctness constraints on emission order (Tile serializes
            # reads after writes in program order):
            #  - K(p, c) is read by attn(p, qc, kt=4c..4c+3) for EVERY qc,
            #    so all 4 K chunks must be emitted before (or early inside
            #    via fillers) pair p's first q-chunk.
            #  - Q(p, c) is read only by attn(p, qc=c).
            #  - V(s) is read by AV(kt=s), emitted at loop slot s+AV_LAG.
            # Everything not needed immediately rides as a filler unit so
            # the scalar engine's exp stream starts ~5us in.
            emit_qk_unit(0, 0, "q")
            emit_qk_unit(0, 0, "k")
            emit_qk_unit(0, 1, "q")
            for s in range(3):
                emit_v_unit(s)

            qk = lambda p, c, w: (lambda: emit_qk_unit(p, c, w))
            vu = lambda st: (lambda: emit_v_unit(st))
            pj = lambda q, e: (lambda: emit_proj_unit(q, e))

            # Fillers ride in the PE slack of the ACT-bound kt loops.
            # Ordering constraints: K(p, c) before pair p's first chunk
            # reads key-tile 4c; Q(p, c) before chunk (p, qc=c); V(s) no
            # later than slot s + AV_LAG - 1 of (0, 0).
            filler_sched = {
                (0, 0): [qk(0, 1, "k"), qk(0, 2, "k"), qk(0, 3, "k")]
                        + [vu(s) for s in range(3, 16)],
                (0, 1): [qk(0, 2, "q"), qk(0, 3, "q"), qk(1, 0, "q"),
                         qk(1, 0, "k"), qk(1, 1, "k"), qk(1, 2, "k"),
                         qk(1, 3, "k")],
                (0, 2): [qk(1, 1, "q"), qk(1, 2, "q"), qk(1, 3, "q")],
                (1, 0): [qk(2, 0, "q"), qk(2, 0, "k"), qk(2, 1, "k"),
                         qk(2, 2, "k"), qk(2, 3, "k")],
                (1, 1): [qk(2, 1, "q"), qk(2, 2, "q"), qk(2, 3, "q")],
                (2, 1): [pj(0, e) for e in range(EMBED // 128)],
                (2, 2): [pj(1, e) for e in range(EMBED // 128)],
                (2, 3): [pj(2, e) for e in range(EMBED // 128)],
            }
            for p in range(PAIRS):
                for qc in range(NQC):
                    emit_attn_qc(p, qc, filler_sched.get((p, qc), []))
            for et in range(EMBED // 128):
                emit_proj_unit(NQC - 1, et)

            rep_ctx.__exit__(None, None, None)

    nc.compile()
    return nc


def _get_nc():
    if "nc" not in _CACHE:
        _CACHE["nc"] = _build()
    return _CACHE["nc"]


def kernel(x, W_qkv, W_proj, b_proj):
    from concourse.bass_utils import run_bass_kernel_spmd
    global LAST_RESULTS

    x = np.asarray(x, dtype=np.float32)
    W_qkv = np.asarray(W_qkv, dtype=np.float32)
    W_proj = np.asarray(W_proj, dtype=np.float32)
    b_proj = np.asarray(b_proj, dtype=np.float32)

    ins = []
    for c in range(N_CORES):
        b, g = divmod(c, 2)
        cols = slice(g * HPC * HD, (g + 1) * HPC * HD)
        ins.append({
            "xt": np.ascontiguousarray(x[b].T).astype(np.float16),
            "wq": np.ascontiguousarray(
                W_qkv[:, 0 * EMBED:1 * EMBED][:, cols]).astype(np.float16),
            "wk": np.ascontiguousarray(
                W_qkv[:, 1 * EMBED:2 * EMBED][:, cols]).astype(np.float16),
            "wv": np.ascontiguousarray(
                W_qkv[:, 2 * EMBED:3 * EMBED][:, cols]).astype(np.float16),
            "wp": np.ascontiguousarray(W_proj[cols, :]).astype(np.float16),
        })

    nc = _get_nc()
    res = run_bass_kernel_spmd(nc, ins, core_ids=list(range(N_CORES)))
    LAST_RESULTS = res

    out = np.empty((B, S, EMBED), np.float32)
    for b in range(B):
        acc = (res.results[2 * b]["o"].astype(np.float32)
               + res.results[2 * b + 1]["o"].astype(np.float32))
        out[b] = acc.T + b_proj
    return out
